# revision 1
# baseline (speedup 1.0000x reference)
"""DeepSeek sparse attention on 8 Trainium2 NeuronCores (Bass/Tile).

Strategy (3 SPMD launches, column/head-parallel, float32r matmuls):

  L1  (column-parallel): each core computes a 256-column slice of the three
      projections, emitted transposed: qT/kT/vT slices (256, S) from
      hidden^T (resident in SBUF) and the core's weight column slice.
  host: concat slices -> q_lin^T, k_lin^T, v_lin^T (H, S).
  L2  (indexer-head-parallel): core c owns indexer head c. Computes
      qp_c^T, kp_c^T (256, S) from full q_lin^T / k_lin^T, then
      rel_c[q] = sum_k relu(qp_c[q] . kp_c[k]) via PE + fused relu-accum.
  host: rel = sum_c w_c * rel_c * exp(-T); top-1024 keys -> selected mask;
      hi[k] = selected ? BIG : k + LOCAL_WINDOW (fp16 threshold vector).
  L3  (attention-head-parallel): core c owns attention heads 2c, 2c+1.
      scores^T per head via PE (f32r), exp via ACT (fp16), causal/local/
      selected masking via two fused iota-compare-multiply DVE ops,
      denominator via ones-matmul, normalize, out rows = ao @ Wo[head rows]
      -> per-core partial (S, H).
  host: out = sum_c partial_c.

Matmuls run as float32r (full PE rate at N>=512, ~1.5e-4 rel err).
"""

import math

import numpy as np

import concourse.bass as bass
import concourse.mybir as mybir
from concourse import bacc
from concourse.tile import TileContext
from concourse.masks import make_identity
from concourse.bass_utils import run_bass_kernel_spmd

# Problem constants (hardcoded per contract)
HIDDEN = 2048
NUM_HEADS = 16
HEAD_DIM = 128
NUM_IND_HEADS = 8
IND_DIM = HIDDEN // NUM_IND_HEADS  # 256
MAX_SELECTED = 1024
LOCAL_WINDOW = 512
N_CORES = 8

F32 = mybir.dt.float32
F32R = mybir.dt.float32r
F16 = mybir.dt.float16
BF16 = mybir.dt.bfloat16
FP32 = np.float32

_TRACE = {"on": False, "exec_ns": []}


def _bc(ap):
    return ap.bitcast(F32R)


def build_l1(S=2048, H=HIDDEN, CS=HIDDEN // N_CORES):
    """Per-core: qT/kT/vT (CS, S) = (W[:, cols].T @ hidden.T) for 3 weights."""
    nc = bacc.Bacc("TRN2", target_bir_lowering=False, debug=False)
    HT, MC, NQ = H // 128, CS // 128, S // 512
    hidT = nc.dram_tensor("hidT", [H, S], F32R, kind="ExternalInput")
    wq = nc.dram_tensor("wq", [H, CS], F32R, kind="ExternalInput")
    wk = nc.dram_tensor("wk", [H, CS], F32R, kind="ExternalInput")
    wv = nc.dram_tensor("wv", [H, CS], F32R, kind="ExternalInput")
    qT = nc.dram_tensor("qT", [CS, S], F32, kind="ExternalOutput")
    kT = nc.dram_tensor("kT", [CS, S], F32, kind="ExternalOutput")
    vT = nc.dram_tensor("vT", [CS, S], F32, kind="ExternalOutput")

    with TileContext(nc) as tc:
        with (
            tc.tile_pool(name="hid", bufs=1) as hpool,
            tc.tile_pool(name="wt", bufs=4) as wpool,
            tc.tile_pool(name="ev", bufs=4) as opool,
            tc.tile_pool(name="ps", bufs=2, space="PSUM") as pspool,
        ):
            # hidden^T resident, loaded as 8 chunks of 2 k-strips so the first
            # matmuls only wait on chunk 0 (~2 MB), not the full 16 MB.
            G = 8
            TG = HT // G

            def load_hidc(g):
                hc = hpool.tile([128, TG * S], F32R, name=f"hidc{g}")
                nc.sync.dma_start(
                    out=hc.rearrange("p (t s) -> p t s", t=TG),
                    in_=hidT[g * TG * 128:(g + 1) * TG * 128, :].rearrange(
                        "(t p) s -> p t s", p=128
                    ),
                )
                return hc

            def load_wres(wdram):
                # weight column-slice resident: one 2 MB DMA per projection.
                wr = wpool.tile([128, HT * CS], F32R, tag="wres", name="wres")
                nc.sync.dma_start(
                    out=wr.rearrange("p (t c) -> p t c", t=HT),
                    in_=wdram.rearrange("(t p) c -> p t c", p=128),
                )
                return wr

            hidc = [load_hidc(0)]
            wres = {wq.name: load_wres(wq)}
            hidc += [load_hidc(g) for g in range(1, G)]
            wres[wk.name] = load_wres(wk)
            wres[wv.name] = load_wres(wv)

            for wdram, odram in ((wq, qT), (wk, kT), (wv, vT)):
                wr = wres[wdram.name]
                for mc in range(MC):
                    psums = [
                        pspool.tile([128, 512], F32, tag=f"ps{qc}", name=f"ps{qc}")
                        for qc in range(NQ)
                    ]
                    for t in range(HT):
                        lhsT = wr[:, t * CS + mc * 128: t * CS + mc * 128 + 128]
                        rhs_tile = hidc[t // TG]
                        tl = t % TG
                        for qc in range(NQ):
                            nc.tensor.matmul(
                                psums[qc], lhsT,
                                rhs_tile[:, tl * S + qc * 512: tl * S + qc * 512 + 512],
                                start=(t == 0), stop=(t == HT - 1),
                            )
                    for qc in range(NQ):
                        ot = opool.tile([128, 512], F32, tag="ot", name="ot")
                        nc.scalar.copy(ot, psums[qc])
                        nc.sync.dma_start(
                            out=odram[mc * 128:(mc + 1) * 128, qc * 512:(qc + 1) * 512],
                            in_=ot,
                        )
    nc.compile()
    return nc


def build_l2(S=2048, H=HIDDEN, D=IND_DIM):
    """Per-core (indexer head c): rel_c[q] = sum_k relu(qp_c[q] . kp_c[k])."""
    nc = bacc.Bacc("TRN2", target_bir_lowering=False, debug=False)
    HT, DC, NQ, QT = H // 128, D // 128, S // 512, S // 128
    qTd = nc.dram_tensor("qT", [H, S], F32R, kind="ExternalInput")
    kTd = nc.dram_tensor("kT", [H, S], F32R, kind="ExternalInput")
    wqi = nc.dram_tensor("wqi", [H, D], F32R, kind="ExternalInput")
    wki = nc.dram_tensor("wki", [H, D], F32R, kind="ExternalInput")
    rel = nc.dram_tensor("rel", [S], F32, kind="ExternalOutput")

    with TileContext(nc) as tc:
        with (
            tc.tile_pool(name="strip", bufs=3) as spool,
            tc.tile_pool(name="wstrip", bufs=3) as wpool,
            tc.tile_pool(name="proj", bufs=1) as ppool,
            tc.tile_pool(name="scr", bufs=3) as scpool,
            tc.tile_pool(name="rc", bufs=2) as rcpool,
            tc.tile_pool(name="rm", bufs=1) as rmpool,
            tc.tile_pool(name="ps", bufs=1, space="PSUM") as pspool,
        ):
            qpt = [ppool.tile([128, S], F32R, name=f"qpt{mc}") for mc in range(DC)]
            kpt = [ppool.tile([128, S], F32R, name=f"kpt{mc}") for mc in range(DC)]
            wires = {}
            for wd in (wqi, wki):
                wr = wpool.tile([128, HT * D], F32R, tag="wires", name="wires")
                nc.sync.dma_start(
                    out=wr.rearrange("p (t c) -> p t c", t=HT),
                    in_=wd.rearrange("(t p) c -> p t c", p=128),
                )
                wires[wd.name] = wr
            for xTd, wd, dst in ((qTd, wqi, qpt), (kTd, wki, kpt)):
                wr = wires[wd.name]
                psq = [
                    pspool.tile([128, 512], F32, tag=f"m{i}", name=f"m{i}")
                    for i in range(DC * NQ)
                ]
                for t in range(HT):
                    xs = spool.tile([128, S], F32R, tag="xs", name="xs")
                    nc.sync.dma_start(out=xs, in_=xTd[t * 128:(t + 1) * 128, :])
                    for mc in range(DC):
                        for qc in range(NQ):
                            nc.tensor.matmul(
                                psq[mc * NQ + qc],
                                wr[:, t * D + mc * 128: t * D + mc * 128 + 128],
                                xs[:, qc * 512:(qc + 1) * 512],
                                start=(t == 0), stop=(t == HT - 1),
                            )
                for mc in range(DC):
                    for qc in range(NQ):
                        nc.scalar.copy(
                            dst[mc][:, qc * 512:(qc + 1) * 512], psq[mc * NQ + qc]
                        )
            relmat = rmpool.tile([128, QT], F32, name="relmat")
            for qt in range(QT):
                relcols = rcpool.tile([128, NQ], F32, tag="relcols", name="relcols")
                spss = [
                    pspool.tile([128, 512], F32, tag=f"m{kc}", name="sps")
                    for kc in range(NQ)
                ]
                for d in range(DC):
                    for kc in range(NQ):
                        nc.tensor.matmul(
                            spss[kc],
                            qpt[d][:, qt * 128:(qt + 1) * 128],
                            kpt[d][:, kc * 512:(kc + 1) * 512],
                            start=(d == 0), stop=(d == DC - 1),
                        )
                for kc in range(NQ):
                    scratch = scpool.tile([128, 512], F16, tag="scratch", name="scratch")
                    nc.scalar.activation(
                        scratch, spss[kc], mybir.ActivationFunctionType.Relu,
                        accum_out=relcols[:, kc:kc + 1],
                    )
                nc.vector.tensor_reduce(
                    relmat[:, qt:qt + 1], relcols, axis=mybir.AxisListType.X,
                    op=mybir.AluOpType.add,
                )
            nc.sync.dma_start(
                out=rel.rearrange("(t p) -> p t", p=128), in_=relmat
            )
    nc.compile()
    return nc


def build_l3(S=2048, H=HIDDEN, NHC=NUM_HEADS // N_CORES, HD=HEAD_DIM,
             window=LOCAL_WINDOW):
    """Per-core (attention heads): partial (S, H) = sum_h softmax-attn @ Wo rows."""
    nc = bacc.Bacc("TRN2", target_bir_lowering=False, debug=False)
    KC, NQ, QT, OCC = S // 128, S // 512, S // 128, H // 512
    WT = window // 128  # local window in k-tiles
    qTh = nc.dram_tensor("qTh", [NHC * HD, S], F32R, kind="ExternalInput")
    kTh = nc.dram_tensor("kTh", [NHC * HD, S], F32R, kind="ExternalInput")
    vTh = nc.dram_tensor("vTh", [NHC * HD, S], F32, kind="ExternalInput")
    woh = nc.dram_tensor("woh", [NHC * HD, H], F32R, kind="ExternalInput")
    kidx = nc.dram_tensor("kidx", [S], F16, kind="ExternalInput")
    hivec = nc.dram_tensor("hivec", [S], F16, kind="ExternalInput")
    selv = nc.dram_tensor("selv", [S], F16, kind="ExternalInput")
    onesrow = nc.dram_tensor("onesrow", [128], F32R, kind="ExternalInput")
    part = nc.dram_tensor("part", [S, H], F32, kind="ExternalOutput")

    scale = 1.0 / math.sqrt(HD)
    AF = mybir.ActivationFunctionType
    OP = mybir.AluOpType

    with TileContext(nc) as tc:
        with (
            tc.tile_pool(name="const", bufs=1) as cpool,
            tc.tile_pool(name="qk", bufs=1) as qkpool,
            tc.tile_pool(name="vt", bufs=2) as vtpool,
            tc.tile_pool(name="vh", bufs=1) as vhpool,
            tc.tile_pool(name="vsl", bufs=1) as vslpool,
            tc.tile_pool(name="et", bufs=2) as etpool,
            tc.tile_pool(name="aon", bufs=1) as aopool,
            tc.tile_pool(name="dr", bufs=2) as drpool,
            tc.tile_pool(name="ev", bufs=4) as evpool,
            tc.tile_pool(name="ps", bufs=1, space="PSUM") as pspool,
        ):
            iota = cpool.tile([128, S], F16, name="iota")
            nc.gpsimd.iota(
                iota, pattern=[[1, S]], base=0, channel_multiplier=0,
                allow_small_or_imprecise_dtypes=True,
            )
            ones = cpool.tile([128, 1], F16, name="ones")
            nc.vector.memset(ones, 1.0)
            ident = cpool.tile([128, 128], F32, name="ident")
            make_identity(nc, ident)
            kvec = cpool.tile([128, KC], F16, name="kvec")
            nc.sync.dma_start(out=kvec, in_=kidx.rearrange("(t p) -> p t", p=128))
            hvec = cpool.tile([128, KC], F16, name="hvec")
            nc.sync.dma_start(out=hvec, in_=hivec.rearrange("(t p) -> p t", p=128))
            svec = cpool.tile([128, KC], F16, name="svec")
            nc.sync.dma_start(out=svec, in_=selv.rearrange("(t p) -> p t", p=128))
            svec32 = cpool.tile([128, KC], F32, name="svec32")
            nc.vector.tensor_copy(svec32, svec)
            ones1 = cpool.tile([1, 128], F32R, name="ones1")
            nc.sync.dma_start(out=ones1, in_=onesrow[None, :])

            # head-0 working set first so PE can start early; wo weights last.
            vts0 = vtpool.tile([128, S], F32, tag="vts", name="vts")
            nc.sync.dma_start(out=vts0, in_=vTh[0:HD, :])
            qsb, ksb = [], []
            for h in range(NHC):
                q = qkpool.tile([128, S], F32R, name=f"qsb{h}")
                nc.sync.dma_start(out=q, in_=qTh[h * HD:(h + 1) * HD, :])
                qsb.append(q)
                k = qkpool.tile([128, S], F32R, name=f"ksb{h}")
                nc.sync.dma_start(out=k, in_=kTh[h * HD:(h + 1) * HD, :])
                ksb.append(k)

            aon = [aopool.tile([128, S], F32R, name=f"aon{h}") for h in range(NHC)]
            vhf = [vhpool.tile([128, S], F16, name=f"vhf{h}") for h in range(NHC)]

            for h in range(NHC):
                if h == 0:
                    vts = vts0
                else:
                    vts = vtpool.tile([128, S], F32, tag="vts", name="vts")
                    nc.sync.dma_start(out=vts, in_=vTh[h * HD:(h + 1) * HD, :])
                for kc in range(KC):
                    tp = pspool.tile([128, 128], F32, tag="sc", bufs=3, name="tp")
                    nc.tensor.transpose(tp, vts[:, kc * 128:(kc + 1) * 128], ident)
                    nc.scalar.copy(vhf[h][:, kc * 128:(kc + 1) * 128], tp)
                # v pre-multiplied by the selected mask: beyond-local tiles use
                # it as the stationary operand, making masking free there.
                vsl = vslpool.tile([128, S], F16, name=f"vsl{h}")
                for kc in range(KC):
                    nc.vector.tensor_scalar_mul(
                        vsl[:, kc * 128:(kc + 1) * 128],
                        vhf[h][:, kc * 128:(kc + 1) * 128],
                        svec32[:, kc:kc + 1],
                    )
                # kc-outer: stationary operands (k tile, v tile) reused across
                # the q chunks; av/den accumulate per q chunk across kc.
                avp = [
                    pspool.tile([128, 512], F32, tag=f"av{qc}", bufs=1,
                                name=f"av{qc}")
                    for qc in range(NQ)
                ]
                den128 = pspool.tile([128, 512], F32, tag="den", bufs=1,
                                     name="den128")
                ets = {}
                for kc in range(KC):
                    k0 = kc * 128
                    qcs = [qc for qc in range(NQ) if k0 <= qc * 512 + 511]
                    far = {qc: qc * 512 > k0 + 127 + window for qc in qcs}
                    for qc in qcs:
                        q0 = qc * 512
                        q1 = q0 + 511
                        sps = pspool.tile([128, 512], F32, tag="sc", bufs=3,
                                          name="sps")
                        nc.tensor.matmul(
                            sps, ksb[h][:, kc * 128:(kc + 1) * 128],
                            qsb[h][:, q0:q0 + 512], start=True, stop=True,
                        )
                        et = etpool.tile([128, 512], F16, tag=f"et{qc}",
                                         name=f"et{qc}")
                        ets[qc] = et
                        nc.scalar.activation(et, sps, AF.Exp, scale=scale)
                        if far[qc]:
                            continue  # sel-mask folded into vsl/svec operands
                        if q0 < k0 + 128:
                            # causal: zero where q < k (iota - k < 0)
                            nc.gpsimd.affine_select(
                                out=et, in_=et, compare_op=OP.is_ge, fill=0.0,
                                base=q0 - k0, channel_multiplier=-1,
                                pattern=[[1, 512]],
                            )
                        if q1 > k0 + window:
                            nc.vector.scalar_tensor_tensor(
                                et, iota[:, q0:q0 + 512], hvec[:, kc:kc + 1], et,
                                op0=OP.is_le, op1=OP.mult,
                            )
                    for qc in qcs:
                        lhs_av = vsl if far[qc] else vhf[h]
                        nc.tensor.matmul(
                            avp[qc], lhs_av[:, kc * 128:(kc + 1) * 128], ets[qc],
                            start=(kc == 0), stop=(kc == (qc * 512 + 511) // 128),
                        )
                    for qc in qcs:
                        lhs_den = svec[:, kc:kc + 1] if far[qc] else ones
                        nc.tensor.matmul(
                            den128[32 * qc:32 * qc + 1, :], lhs_den, ets[qc],
                            start=(kc == 0), stop=(kc == (qc * 512 + 511) // 128),
                            tile_position=(0, 32 * qc),
                        )
                # denominators -> reciprocals -> broadcast -> normalize
                for qc in range(NQ):
                    q0 = qc * 512
                    dq = drpool.tile([1, 512], F32, tag=f"dq{qc}", name=f"dq{qc}")
                    nc.scalar.copy(dq, den128[32 * qc:32 * qc + 1, :])
                    rq = drpool.tile([1, 512], F32, tag=f"rq{qc}", name=f"rq{qc}")
                    rs = drpool.tile([1, 512], F32, tag=f"rs{qc}", name=f"rs{qc}")
                    nc.vector.reciprocal_approx_accurate(rq, dq, rs)
                    rcq = drpool.tile([1, 512], F32R, tag=f"rcq{qc}",
                                      name=f"rcq{qc}")
                    nc.vector.tensor_copy(rcq, rq)
                    rb = pspool.tile([128, 512], F32, tag="sc", bufs=3, name="rb")
                    nc.tensor.matmul(rb, ones1, rcq, start=True, stop=True)
                    rbs = drpool.tile([128, 512], F32, tag="rbs", name="rbs")
                    nc.scalar.copy(rbs, rb)
                    nc.vector.scalar_tensor_tensor(
                        aon[h][:, q0:q0 + 512], rbs, 1.0, avp[qc],
                        op0=OP.mult, op1=OP.mult,
                    )
            wsb = []
            for h in range(NHC):
                w = qkpool.tile([128, H], F32R, name=f"wsb{h}")
                nc.sync.dma_start(out=w, in_=woh[h * HD:(h + 1) * HD, :])
                wsb.append(w)
            for qt in range(QT):
                wops = [
                    pspool.tile([128, 512], F32, tag=f"av{oc}", bufs=1,
                                name=f"wops{oc}")
                    for oc in range(OCC)
                ]
                for h in range(NHC):
                    for oc in range(OCC):
                        nc.tensor.matmul(
                            wops[oc], aon[h][:, qt * 128:(qt + 1) * 128],
                            wsb[h][:, oc * 512:(oc + 1) * 512],
                            start=(h == 0), stop=(h == NHC - 1),
                        )
                for oc in range(OCC):
                    ot = evpool.tile([128, 512], F32, tag="ot", name="ot")
                    nc.vector.tensor_copy(ot, wops[oc])
                    nc.sync.dma_start(
                        out=part[qt * 128:(qt + 1) * 128, oc * 512:(oc + 1) * 512],
                        in_=ot,
                    )
    nc.compile()
    return nc


_CACHE = {}


def _get(name, builder, *args):
    key = (name,) + args
    if key not in _CACHE:
        _CACHE[key] = builder(*args)
    return _CACHE[key]


def _run(nc, in_maps):
    res = run_bass_kernel_spmd(
        nc, in_maps, core_ids=list(range(N_CORES)), trace=_TRACE["on"]
    )
    if _TRACE["on"] and res.exec_time_ns is not None:
        _TRACE["exec_ns"].append(res.exec_time_ns)
    return res.results


def kernel(hidden_states, Wq, Wk, Wv, Wo, Wq_ind, Wk_ind, head_weights,
           temperature_param):
    hidden_states = np.asarray(hidden_states, dtype=FP32)
    Wq, Wk, Wv, Wo = (np.asarray(a, dtype=FP32) for a in (Wq, Wk, Wv, Wo))
    Wq_ind = np.asarray(Wq_ind, dtype=FP32)
    Wk_ind = np.asarray(Wk_ind, dtype=FP32)
    head_weights = np.asarray(head_weights, dtype=FP32)
    temp = float(np.asarray(temperature_param))

    B, S, H = hidden_states.shape
    assert B == 1 and H == HIDDEN
    CS = H // N_CORES
    hidT = np.ascontiguousarray(hidden_states[0].T)

    # ---- L1: projections, column-parallel ----
    nc1 = _get("l1", build_l1, S, H, CS)
    in1 = [
        {
            "hidT": hidT,
            "wq": np.ascontiguousarray(Wq[:, c * CS:(c + 1) * CS]),
            "wk": np.ascontiguousarray(Wk[:, c * CS:(c + 1) * CS]),
            "wv": np.ascontiguousarray(Wv[:, c * CS:(c + 1) * CS]),
        }
        for c in range(N_CORES)
    ]
    r1 = _run(nc1, in1)
    qTf = np.concatenate([r["qT"] for r in r1], axis=0)
    kTf = np.concatenate([r["kT"] for r in r1], axis=0)
    vTf = np.concatenate([r["vT"] for r in r1], axis=0)

    # ---- L2: lightning indexer, head-parallel ----
    D = IND_DIM
    nc2 = _get("l2", build_l2, S, H, D)
    in2 = [
        {
            "qT": qTf,
            "kT": kTf,
            "wqi": np.ascontiguousarray(Wq_ind[:, c * D:(c + 1) * D]),
            "wki": np.ascontiguousarray(Wk_ind[:, c * D:(c + 1) * D]),
        }
        for c in range(N_CORES)
    ]
    r2 = _run(nc2, in2)
    rel = np.zeros(S, dtype=np.float64)
    for c in range(N_CORES):
        rel += float(head_weights[c]) * r2[c]["rel"].astype(np.float64)
    # exp(-temp) scaling is monotone; irrelevant for top-k selection.

    k_sel = min(MAX_SELECTED, S)
    top_idx = np.argpartition(-rel, k_sel - 1)[:k_sel]
    selected = np.zeros(S, dtype=bool)
    selected[top_idx] = True

    # ---- L3: masked attention + output projection, head-parallel ----
    BIG = float(2 * S + 1024)
    hi = np.where(selected, BIG, np.arange(S, dtype=np.float64) + LOCAL_WINDOW)
    hi = hi.astype(np.float16)
    kidx = np.arange(S, dtype=np.float16)
    selv = selected.astype(np.float16)
    NHC = NUM_HEADS // N_CORES
    RW = NHC * HEAD_DIM
    nc3 = _get("l3", build_l3, S, H, NHC, HEAD_DIM, LOCAL_WINDOW)
    in3 = [
        {
            "qTh": np.ascontiguousarray(qTf[c * RW:(c + 1) * RW]),
            "kTh": np.ascontiguousarray(kTf[c * RW:(c + 1) * RW]),
            "vTh": np.ascontiguousarray(vTf[c * RW:(c + 1) * RW]),
            "woh": np.ascontiguousarray(Wo[c * RW:(c + 1) * RW]),
            "kidx": kidx,
            "hivec": hi,
            "selv": selv,
            "onesrow": np.ones(128, dtype=np.float32),
        }
        for c in range(N_CORES)
    ]
    r3 = _run(nc3, in3)
    out = r3[0]["part"]
    for c in range(1, N_CORES):
        out = out + r3[c]["part"]
    return out.reshape(B, S, H).astype(np.float32)



# revision 2
# speedup vs baseline: 1.1815x; 1.1815x over previous
"""DeepSeek sparse attention on 8 Trainium2 NeuronCores (Bass/Tile).

Strategy (2 SPMD launches, head-parallel, f32r indexer + fp16 attention):

  host: fuse indexer weights through the attention projections:
      Wfq = Wq @ Wq_ind, Wfk = Wk @ Wk_ind  (so the indexer reads
      hidden directly and needs no q_lin/k_lin round trip).
  A   (fused projections + indexer): core c keeps hidden^T resident in
      SBUF (one 16.8 MB stream) and runs five accumulation passes:
      wfq -> qp_c^T, wfk -> kp_c^T (indexer head c, f32r), then the
      relu(qp.kp) score reduction -> rel_c, then wq/wk/wv -> the
      column slices qT/kT (fp16 out) / vT (f32 out) = exactly the
      core's own 2 attention heads.
  host: rel = sum_c w_c * rel_c; top-1024 keys -> selected mask ->
      hi[k] threshold vector (fp16).
  B   (attention, fp16): core c computes softmax attention for its 2
      heads from its own qT/kT/vT slices, masked causal/local/selected
      via iota-compare DVE ops, then partial out rows = ao @ Wo[rows]
      -> per-core partial (S, H) in fp16.
  host: out = sum_c partial_c.

The indexer path stays f32r end-to-end: the top-k boundary gaps are
~1e-5 relative, and selection flips cost ~1e-2 output error each.
The attention path is fp16 (storage) with f32 PE accumulation.
"""

import math

import numpy as np

import concourse.bass as bass
import concourse.mybir as mybir
from concourse import bacc
from concourse.tile import TileContext
from concourse.masks import make_identity
from concourse.bass_utils import run_bass_kernel_spmd

# Problem constants (hardcoded per contract)
HIDDEN = 2048
NUM_HEADS = 16
HEAD_DIM = 128
NUM_IND_HEADS = 8
IND_DIM = HIDDEN // NUM_IND_HEADS  # 256
MAX_SELECTED = 1024
LOCAL_WINDOW = 512
N_CORES = 8

F32 = mybir.dt.float32
F32R = mybir.dt.float32r
F16 = mybir.dt.float16
FP32 = np.float32

_TRACE = {"on": False, "exec_ns": []}


def build_fused(S=2048, H=HIDDEN, CS=HIDDEN // N_CORES, D=IND_DIM):
    """Launch A: hidden^T resident; 5 projection passes + indexer scores."""
    nc = bacc.Bacc("TRN2", target_bir_lowering=False, debug=False)
    HT, NQ, QT, DC, MC = H // 128, S // 512, S // 128, D // 128, CS // 128
    hidT = nc.dram_tensor("hidT", [H, S], F32R, kind="ExternalInput")
    wq = nc.dram_tensor("wq", [H, CS], F32R, kind="ExternalInput")
    wk = nc.dram_tensor("wk", [H, CS], F32R, kind="ExternalInput")
    wv = nc.dram_tensor("wv", [H, CS], F32R, kind="ExternalInput")
    wfq = nc.dram_tensor("wfq", [H, D], F32R, kind="ExternalInput")
    wfk = nc.dram_tensor("wfk", [H, D], F32R, kind="ExternalInput")
    qT = nc.dram_tensor("qT", [CS, S], F16, kind="ExternalOutput")
    kT = nc.dram_tensor("kT", [CS, S], F16, kind="ExternalOutput")
    vT = nc.dram_tensor("vT", [CS, S], F32, kind="ExternalOutput")
    rel = nc.dram_tensor("rel", [S], F32, kind="ExternalOutput")

    AF = mybir.ActivationFunctionType

    with TileContext(nc) as tc:
        with (
            tc.tile_pool(name="hid", bufs=1) as hpool,
            tc.tile_pool(name="wt", bufs=4) as wpool,
            tc.tile_pool(name="proj", bufs=1) as ppool,
            tc.tile_pool(name="ev", bufs=6) as opool,
            tc.tile_pool(name="scr", bufs=2) as scpool,
            tc.tile_pool(name="rc", bufs=2) as rcpool,
            tc.tile_pool(name="rm", bufs=1) as rmpool,
            tc.tile_pool(name="ps", bufs=1, space="PSUM") as pspool,
        ):
            # hidden^T resident, loaded as 8 chunks of 2 k-strips so the first
            # matmuls only wait on chunk 0 (~2 MB), not the full 16 MB.
            G = 8
            TG = HT // G

            def load_hidc(g):
                hc = hpool.tile([128, TG * S], F32R, name=f"hidc{g}")
                nc.sync.dma_start(
                    out=hc.rearrange("p (t s) -> p t s", t=TG),
                    in_=hidT[g * TG * 128:(g + 1) * TG * 128, :].rearrange(
                        "(t p) s -> p t s", p=128
                    ),
                )
                return hc

            hidc = [load_hidc(g) for g in range(G)]

            def hstrip(t):
                return hidc[t // TG][:, (t % TG) * S:(t % TG) * S + S]

            qpt = [ppool.tile([128, S], F32R, name=f"qpt{i}") for i in range(DC)]
            kpt = [ppool.tile([128, S], F32R, name=f"kpt{i}") for i in range(DC)]

            def proj_pass(wd, sink):
                psq = [
                    pspool.tile([128, 512], F32, tag=f"m{i}", name=f"m{i}")
                    for i in range(MC * NQ)
                ]
                for t in range(HT):
                    w = wpool.tile([128, CS], F32R, tag="ws", name="ws")
                    nc.sync.dma_start(out=w, in_=wd[t * 128:(t + 1) * 128, :])
                    rhs = hstrip(t)
                    for mc in range(MC):
                        for qc in range(NQ):
                            nc.tensor.matmul(
                                psq[mc * NQ + qc],
                                w[:, mc * 128:(mc + 1) * 128],
                                rhs[:, qc * 512:(qc + 1) * 512],
                                start=(t == 0), stop=(t == HT - 1),
                            )
                for mc in range(MC):
                    for qc in range(NQ):
                        sink(mc, qc, psq[mc * NQ + qc])

            def sink_proj(dst):
                def s(mc, qc, ps):
                    nc.scalar.copy(dst[mc][:, qc * 512:(qc + 1) * 512], ps)
                return s

            def sink_out(odram, dt, use_act):
                def s(mc, qc, ps):
                    ot = opool.tile([128, 512], dt, tag=f"ot{dt}", name="ot")
                    if use_act:
                        nc.scalar.copy(ot, ps)
                    else:
                        nc.vector.tensor_copy(ot, ps)
                    nc.sync.dma_start(
                        out=odram[mc * 128:(mc + 1) * 128,
                                  qc * 512:(qc + 1) * 512],
                        in_=ot,
                    )
                return s

            # indexer projections first so the score phase can start early;
            # attention projections after (their ACT/DVE copies overlap scores)
            proj_pass(wfq, sink_proj(qpt))
            proj_pass(wfk, sink_proj(kpt))

            # indexer scores: rel_c[q] = sum_k relu(qp_c[q] . kp_c[k])
            relmat = rmpool.tile([128, QT], F32, name="relmat")
            for qt in range(QT):
                spss = [
                    pspool.tile([128, 512], F32, tag=f"m{kc}", name="sps")
                    for kc in range(NQ)
                ]
                for dcc in range(DC):
                    for kc in range(NQ):
                        nc.tensor.matmul(
                            spss[kc],
                            qpt[dcc][:, qt * 128:(qt + 1) * 128],
                            kpt[dcc][:, kc * 512:(kc + 1) * 512],
                            start=(dcc == 0), stop=(dcc == DC - 1),
                        )
                relcols = rcpool.tile([128, NQ + 1], F32, tag="relcols",
                                      name="relcols")
                for kc in (0, 1):  # ACT half: fused relu + free-axis accum
                    scratch = scpool.tile([128, 512], F16, tag=f"scr{kc}",
                                          name="scratch")
                    nc.scalar.activation(
                        scratch, spss[kc], AF.Relu,
                        accum_out=relcols[:, kc:kc + 1],
                    )
                for kc in (2, 3):  # DVE half: relu then reduce
                    scratch = scpool.tile([128, 512], F16, tag=f"scr{kc}",
                                          name="scratch")
                    nc.vector.tensor_scalar_max(scratch, spss[kc], 0.0)
                    nc.vector.tensor_reduce(
                        relcols[:, kc:kc + 1], scratch,
                        axis=mybir.AxisListType.X, op=mybir.AluOpType.add,
                    )
                nc.vector.tensor_reduce(
                    relmat[:, qt:qt + 1], relcols[:, 0:NQ],
                    axis=mybir.AxisListType.X, op=mybir.AluOpType.add,
                )
            nc.sync.dma_start(
                out=rel.rearrange("(t p) -> p t", p=128), in_=relmat
            )

            # attention projections: q/k in fp16 for launch B, v in f32
            proj_pass(wq, sink_out(qT, F16, True))
            proj_pass(wk, sink_out(kT, F16, False))
            proj_pass(wv, sink_out(vT, F32, True))
    nc.compile()
    return nc


def build_attn(S=2048, H=HIDDEN, NHC=NUM_HEADS // N_CORES, HD=HEAD_DIM,
               window=LOCAL_WINDOW):
    """Launch B: per-core (2 heads) masked softmax attention + out-proj."""
    nc = bacc.Bacc("TRN2", target_bir_lowering=False, debug=False)
    KC, NQ, QT, OCC = S // 128, S // 512, S // 128, H // 512
    qTh = nc.dram_tensor("qTh", [NHC * HD, S], F16, kind="ExternalInput")
    kTh = nc.dram_tensor("kTh", [NHC * HD, S], F16, kind="ExternalInput")
    vTh = nc.dram_tensor("vTh", [NHC * HD, S], F32, kind="ExternalInput")
    woh = nc.dram_tensor("woh", [NHC * HD, H], F16, kind="ExternalInput")
    kidx = nc.dram_tensor("kidx", [S], F16, kind="ExternalInput")
    hivec = nc.dram_tensor("hivec", [S], F16, kind="ExternalInput")
    selv = nc.dram_tensor("selv", [S], F16, kind="ExternalInput")
    onesrow = nc.dram_tensor("onesrow", [128], F32R, kind="ExternalInput")
    part = nc.dram_tensor("part", [S, H], F16, kind="ExternalOutput")

    scale = 1.0 / math.sqrt(HD)
    AF = mybir.ActivationFunctionType
    OP = mybir.AluOpType

    with TileContext(nc) as tc:
        with (
            tc.tile_pool(name="const", bufs=1) as cpool,
            tc.tile_pool(name="qk", bufs=1) as qkpool,
            tc.tile_pool(name="vt", bufs=2) as vtpool,
            tc.tile_pool(name="vh", bufs=1) as vhpool,
            tc.tile_pool(name="vsl", bufs=1) as vslpool,
            tc.tile_pool(name="et", bufs=2) as etpool,
            tc.tile_pool(name="aon", bufs=1) as aopool,
            tc.tile_pool(name="dr", bufs=2) as drpool,
            tc.tile_pool(name="ev", bufs=4) as evpool,
            tc.tile_pool(name="ps", bufs=1, space="PSUM") as pspool,
        ):
            iota = cpool.tile([128, S], F16, name="iota")
            nc.gpsimd.iota(
                iota, pattern=[[1, S]], base=0, channel_multiplier=0,
                allow_small_or_imprecise_dtypes=True,
            )
            ones = cpool.tile([128, 1], F16, name="ones")
            nc.vector.memset(ones, 1.0)
            ident = cpool.tile([128, 128], F32, name="ident")
            make_identity(nc, ident)
            kvec = cpool.tile([128, KC], F16, name="kvec")
            nc.sync.dma_start(out=kvec, in_=kidx.rearrange("(t p) -> p t", p=128))
            hvec = cpool.tile([128, KC], F16, name="hvec")
            nc.sync.dma_start(out=hvec, in_=hivec.rearrange("(t p) -> p t", p=128))
            svec = cpool.tile([128, KC], F16, name="svec")
            nc.sync.dma_start(out=svec, in_=selv.rearrange("(t p) -> p t", p=128))
            svec32 = cpool.tile([128, KC], F32, name="svec32")
            nc.vector.tensor_copy(svec32, svec)
            ones1 = cpool.tile([1, 128], F32R, name="ones1")
            nc.sync.dma_start(out=ones1, in_=onesrow[None, :])

            # head-0 working set first so PE can start early; wo weights last.
            vts0 = vtpool.tile([128, S], F32, tag="vts", name="vts")
            nc.sync.dma_start(out=vts0, in_=vTh[0:HD, :])
            qsb, ksb = [], []
            for h in range(NHC):
                q = qkpool.tile([128, S], F16, name=f"qsb{h}")
                nc.sync.dma_start(out=q, in_=qTh[h * HD:(h + 1) * HD, :])
                qsb.append(q)
                k = qkpool.tile([128, S], F16, name=f"ksb{h}")
                nc.sync.dma_start(out=k, in_=kTh[h * HD:(h + 1) * HD, :])
                ksb.append(k)

            aon = [aopool.tile([128, S], F16, name=f"aon{h}") for h in range(NHC)]
            vhf = [vhpool.tile([128, S], F16, name=f"vhf{h}") for h in range(NHC)]

            for h in range(NHC):
                if h == 0:
                    vts = vts0
                else:
                    vts = vtpool.tile([128, S], F32, tag="vts", name="vts")
                    nc.sync.dma_start(out=vts, in_=vTh[h * HD:(h + 1) * HD, :])
                for kc in range(KC):
                    tp = pspool.tile([128, 128], F32, tag="sc", bufs=3, name="tp")
                    nc.tensor.transpose(tp, vts[:, kc * 128:(kc + 1) * 128], ident)
                    nc.scalar.copy(vhf[h][:, kc * 128:(kc + 1) * 128], tp)
                # v pre-multiplied by the selected mask: beyond-local tiles use
                # it as the stationary operand, making masking free there.
                vsl = vslpool.tile([128, S], F16, name=f"vsl{h}")
                for kc in range(KC):
                    nc.vector.tensor_scalar_mul(
                        vsl[:, kc * 128:(kc + 1) * 128],
                        vhf[h][:, kc * 128:(kc + 1) * 128],
                        svec32[:, kc:kc + 1],
                    )
                # kc-outer: stationary operands (k tile, v tile) reused across
                # the q chunks; av/den accumulate per q chunk across kc.
                avp = [
                    pspool.tile([128, 512], F32, tag=f"av{qc}", bufs=1,
                                name=f"av{qc}")
                    for qc in range(NQ)
                ]
                den128 = pspool.tile([128, 512], F32, tag="den", bufs=1,
                                     name="den128")
                ets = {}
                for kc in range(KC):
                    k0 = kc * 128
                    qcs = [qc for qc in range(NQ) if k0 <= qc * 512 + 511]
                    far = {qc: qc * 512 > k0 + 127 + window for qc in qcs}
                    for qc in qcs:
                        q0 = qc * 512
                        q1 = q0 + 511
                        sps = pspool.tile([128, 512], F32, tag="sc", bufs=3,
                                          name="sps")
                        nc.tensor.matmul(
                            sps, ksb[h][:, kc * 128:(kc + 1) * 128],
                            qsb[h][:, q0:q0 + 512], start=True, stop=True,
                        )
                        et = etpool.tile([128, 512], F16, tag=f"et{qc}",
                                         name=f"et{qc}")
                        ets[qc] = et
                        nc.scalar.activation(et, sps, AF.Exp, scale=scale)
                        if far[qc]:
                            continue  # sel-mask folded into vsl/svec operands
                        if q0 < k0 + 128:
                            # causal: zero where q < k (iota - k < 0)
                            nc.gpsimd.affine_select(
                                out=et, in_=et, compare_op=OP.is_ge, fill=0.0,
                                base=q0 - k0, channel_multiplier=-1,
                                pattern=[[1, 512]],
                            )
                        if q1 > k0 + window:
                            nc.vector.scalar_tensor_tensor(
                                et, iota[:, q0:q0 + 512], hvec[:, kc:kc + 1], et,
                                op0=OP.is_le, op1=OP.mult,
                            )
                    for qc in qcs:
                        lhs_av = vsl if far[qc] else vhf[h]
                        nc.tensor.matmul(
                            avp[qc], lhs_av[:, kc * 128:(kc + 1) * 128], ets[qc],
                            start=(kc == 0), stop=(kc == (qc * 512 + 511) // 128),
                        )
                    for qc in qcs:
                        lhs_den = svec[:, kc:kc + 1] if far[qc] else ones
                        nc.tensor.matmul(
                            den128[32 * qc:32 * qc + 1, :], lhs_den, ets[qc],
                            start=(kc == 0), stop=(kc == (qc * 512 + 511) // 128),
                            tile_position=(0, 32 * qc),
                        )
                # denominators -> reciprocals -> broadcast -> normalize
                for qc in range(NQ):
                    q0 = qc * 512
                    dq = drpool.tile([1, 512], F32, tag=f"dq{qc}", name=f"dq{qc}")
                    nc.scalar.copy(dq, den128[32 * qc:32 * qc + 1, :])
                    rq = drpool.tile([1, 512], F32, tag=f"rq{qc}", name=f"rq{qc}")
                    rs = drpool.tile([1, 512], F32, tag=f"rs{qc}", name=f"rs{qc}")
                    nc.vector.reciprocal_approx_accurate(rq, dq, rs)
                    rcq = drpool.tile([1, 512], F32R, tag=f"rcq{qc}",
                                      name=f"rcq{qc}")
                    nc.vector.tensor_copy(rcq, rq)
                    rb = pspool.tile([128, 512], F32, tag="sc", bufs=3, name="rb")
                    nc.tensor.matmul(rb, ones1, rcq, start=True, stop=True)
                    rbs = drpool.tile([128, 512], F32, tag="rbs", name="rbs")
                    nc.vector.tensor_copy(rbs, rb)
                    nc.vector.scalar_tensor_tensor(
                        aon[h][:, q0:q0 + 512], rbs, 1.0, avp[qc],
                        op0=OP.mult, op1=OP.mult,
                    )
            wsb = []
            for h in range(NHC):
                w = qkpool.tile([128, H], F16, name=f"wsb{h}")
                nc.sync.dma_start(out=w, in_=woh[h * HD:(h + 1) * HD, :])
                wsb.append(w)
            for qt in range(QT):
                wops = [
                    pspool.tile([128, 512], F32, tag=f"av{oc}", bufs=1,
                                name=f"wops{oc}")
                    for oc in range(OCC)
                ]
                for h in range(NHC):
                    for oc in range(OCC):
                        nc.tensor.matmul(
                            wops[oc], aon[h][:, qt * 128:(qt + 1) * 128],
                            wsb[h][:, oc * 512:(oc + 1) * 512],
                            start=(h == 0), stop=(h == NHC - 1),
                        )
                for oc in range(OCC):
                    ot = evpool.tile([128, 512], F16, tag="ot", name="ot")
                    nc.vector.tensor_copy(ot, wops[oc])
                    nc.sync.dma_start(
                        out=part[qt * 128:(qt + 1) * 128, oc * 512:(oc + 1) * 512],
                        in_=ot,
                    )
    nc.compile()
    return nc


_CACHE = {}


def _get(name, builder, *args):
    key = (name,) + args
    if key not in _CACHE:
        _CACHE[key] = builder(*args)
    return _CACHE[key]


def _run(nc, in_maps):
    res = run_bass_kernel_spmd(
        nc, in_maps, core_ids=list(range(N_CORES)), trace=_TRACE["on"]
    )
    if _TRACE["on"] and res.exec_time_ns is not None:
        _TRACE["exec_ns"].append(res.exec_time_ns)
    return res.results


def kernel(hidden_states, Wq, Wk, Wv, Wo, Wq_ind, Wk_ind, head_weights,
           temperature_param):
    hidden_states = np.asarray(hidden_states, dtype=FP32)
    Wq, Wk, Wv, Wo = (np.asarray(a, dtype=FP32) for a in (Wq, Wk, Wv, Wo))
    Wq_ind = np.asarray(Wq_ind, dtype=FP32)
    Wk_ind = np.asarray(Wk_ind, dtype=FP32)
    head_weights = np.asarray(head_weights, dtype=FP32)

    B, S, H = hidden_states.shape
    assert B == 1 and H == HIDDEN
    CS = H // N_CORES
    D = IND_DIM
    hidT = np.ascontiguousarray(hidden_states[0].T)
    Wfq = Wq @ Wq_ind  # fused indexer weights (f32 host fuse)
    Wfk = Wk @ Wk_ind

    # ---- Launch A: projections + indexer, head-parallel ----
    ncA = _get("A", build_fused, S, H, CS, D)
    inA = [
        {
            "hidT": hidT,
            "wq": np.ascontiguousarray(Wq[:, c * CS:(c + 1) * CS]),
            "wk": np.ascontiguousarray(Wk[:, c * CS:(c + 1) * CS]),
            "wv": np.ascontiguousarray(Wv[:, c * CS:(c + 1) * CS]),
            "wfq": np.ascontiguousarray(Wfq[:, c * D:(c + 1) * D]),
            "wfk": np.ascontiguousarray(Wfk[:, c * D:(c + 1) * D]),
        }
        for c in range(N_CORES)
    ]
    rA = _run(ncA, inA)
    rel = np.zeros(S, dtype=np.float64)
    for c in range(N_CORES):
        rel += float(head_weights[c]) * rA[c]["rel"].astype(np.float64)
    # exp(-temp) scaling is monotone; irrelevant for top-k selection.

    k_sel = min(MAX_SELECTED, S)
    top_idx = np.argpartition(-rel, k_sel - 1)[:k_sel]
    selected = np.zeros(S, dtype=bool)
    selected[top_idx] = True

    # ---- Launch B: masked attention + output projection, head-parallel ----
    BIG = float(2 * S + 1024)
    hi = np.where(selected, BIG, np.arange(S, dtype=np.float64) + LOCAL_WINDOW)
    hi = hi.astype(np.float16)
    kidx = np.arange(S, dtype=np.float16)
    selv = selected.astype(np.float16)
    NHC = NUM_HEADS // N_CORES
    RW = NHC * HEAD_DIM
    ncB = _get("B", build_attn, S, H, NHC, HEAD_DIM, LOCAL_WINDOW)
    inB = [
        {
            "qTh": rA[c]["qT"],
            "kTh": rA[c]["kT"],
            "vTh": rA[c]["vT"],
            "woh": np.ascontiguousarray(Wo[c * RW:(c + 1) * RW]).astype(
                np.float16),
            "kidx": kidx,
            "hivec": hi,
            "selv": selv,
            "onesrow": np.ones(128, dtype=np.float32),
        }
        for c in range(N_CORES)
    ]
    rB = _run(ncB, inB)
    out = rB[0]["part"].astype(np.float32)
    for c in range(1, N_CORES):
        out += rB[c]["part"].astype(np.float32)
    return out.reshape(B, S, H)


# revision 14
# speedup vs baseline: 1.2385x; 1.0482x over previous
"""DeepSeek sparse attention on 8 Trainium2 NeuronCores (Bass/Tile).

Strategy (2 SPMD launches, head-parallel, f32r indexer + fp16 attention):

  host: fuse indexer weights through the attention projections:
      Wfq = Wq @ Wq_ind, Wfk = Wk @ Wk_ind  (so the indexer reads
      hidden directly and needs no q_lin/k_lin round trip).
  A   (fused projections + indexer): core c keeps hidden^T resident in
      SBUF (one 16.8 MB stream) and runs five accumulation passes:
      wfq -> qp_c^T, wfk -> kp_c^T (indexer head c, f32r), then the
      relu(qp.kp) score reduction -> rel_c, then wq/wk/wv -> the
      column slices qT/kT (fp16 out) / vT (f32 out) = exactly the
      core's own 2 attention heads.
  host: rel = sum_c w_c * rel_c; top-1024 keys -> selected mask ->
      hi[k] threshold vector (fp16).
  B   (attention, fp16): core c computes softmax attention for its 2
      heads from its own qT/kT/vT slices, masked causal/local/selected
      via iota-compare DVE ops, then partial out rows = ao @ Wo[rows]
      -> per-core partial (S, H) in fp16.
  host: out = sum_c partial_c.

The indexer path stays f32r end-to-end: the top-k boundary gaps are
~1e-5 relative, and selection flips cost ~1e-2 output error each.
The attention path is fp16 (storage) with f32 PE accumulation.
"""

import math

import numpy as np

import concourse.bass as bass
import concourse.mybir as mybir
from concourse import bacc
from concourse.tile import TileContext
from concourse.masks import make_identity
from concourse.bass_utils import run_bass_kernel_spmd

# Problem constants (hardcoded per contract)
HIDDEN = 2048
NUM_HEADS = 16
HEAD_DIM = 128
NUM_IND_HEADS = 8
IND_DIM = HIDDEN // NUM_IND_HEADS  # 256
MAX_SELECTED = 1024
LOCAL_WINDOW = 512
N_CORES = 8

F32 = mybir.dt.float32
F32R = mybir.dt.float32r
F16 = mybir.dt.float16
FP32 = np.float32

_TRACE = {"on": False, "exec_ns": []}


def build_fused(S=2048, H=HIDDEN, CS=HIDDEN // N_CORES, D=IND_DIM):
    """Launch A: hidden^T resident; 5 projection passes + indexer scores."""
    nc = bacc.Bacc("TRN2", target_bir_lowering=False, debug=False)
    HT, NQ, QT, DC, MC = H // 128, S // 512, S // 128, D // 128, CS // 128
    hidT = nc.dram_tensor("hidT", [H, S], F32R, kind="ExternalInput")
    wq = nc.dram_tensor("wq", [H, CS], F32R, kind="ExternalInput")
    wk = nc.dram_tensor("wk", [H, CS], F32R, kind="ExternalInput")
    wv = nc.dram_tensor("wv", [H, CS], F32R, kind="ExternalInput")
    wfq = nc.dram_tensor("wfq", [H, D], F32R, kind="ExternalInput")
    wfk = nc.dram_tensor("wfk", [H, D], F32R, kind="ExternalInput")
    qT = nc.dram_tensor("qT", [CS, S], F16, kind="ExternalOutput")
    kT = nc.dram_tensor("kT", [CS, S], F16, kind="ExternalOutput")
    vT = nc.dram_tensor("vT", [CS, S], F16, kind="ExternalOutput")
    rel = nc.dram_tensor("rel", [S], F32, kind="ExternalOutput")

    AF = mybir.ActivationFunctionType

    with TileContext(nc) as tc:
        with (
            tc.tile_pool(name="hid", bufs=1) as hpool,
            tc.tile_pool(name="wres", bufs=1) as wrpool,
            tc.tile_pool(name="wt", bufs=4) as wpool,
            tc.tile_pool(name="proj", bufs=1) as ppool,
            tc.tile_pool(name="ev", bufs=4) as opool,
            tc.tile_pool(name="scr", bufs=1) as scpool,
            tc.tile_pool(name="rc", bufs=2) as rcpool,
            tc.tile_pool(name="rm", bufs=1) as rmpool,
            tc.tile_pool(name="ps", bufs=1, space="PSUM") as pspool,
        ):
            # hidden^T resident, loaded as 8 chunks of 2 k-strips so the first
            # matmuls only wait on chunk 0 (~2 MB), not the full 16 MB.
            G = 8
            TG = HT // G

            def load_hidc(g):
                hc = hpool.tile([128, TG * S], F32R, name=f"hidc{g}")
                nc.sync.dma_start(
                    out=hc.rearrange("p (t s) -> p t s", t=TG),
                    in_=hidT[g * TG * 128:(g + 1) * TG * 128, :].rearrange(
                        "(t p) s -> p t s", p=128
                    ),
                )
                return hc

            def load_wres(wd):
                # indexer weight resident: one 2 MB DMA, issued early so it
                # is not queued behind the 16 MB hidden stream.
                wr = wrpool.tile([128, HT * D], F32R, tag=wd.name, name=wd.name)
                nc.sync.dma_start(
                    out=wr.rearrange("p (t c) -> p t c", t=HT),
                    in_=wd.rearrange("(t p) c -> p t c", p=128),
                )
                return wr

            # DMA issue order: chunk0, wfq, chunks1-3, wfk, chunks4-7 — the
            # first pass's operands land before the bulk of hidden.
            hidc = [load_hidc(0)]
            wfq_r = load_wres(wfq)
            hidc += [load_hidc(g) for g in range(1, 4)]
            wfk_r = load_wres(wfk)
            hidc += [load_hidc(g) for g in range(4, G)]

            def hstrip(t):
                return hidc[t // TG][:, (t % TG) * S:(t % TG) * S + S]

            qpt = [ppool.tile([128, S], F32R, name=f"qpt{i}") for i in range(DC)]
            kpt = [ppool.tile([128, S], F32R, name=f"kpt{i}") for i in range(DC)]

            def proj_pass(wd, wres, sink):
                psq = [
                    pspool.tile([128, 512], F32, tag=f"m{i}", name=f"m{i}")
                    for i in range(MC * NQ)
                ]
                for t in range(HT):
                    if wres is not None:
                        w = wres[:, t * CS:(t + 1) * CS]
                    else:
                        w = wpool.tile([128, CS], F32R, tag="ws", name="ws")
                        nc.sync.dma_start(out=w, in_=wd[t * 128:(t + 1) * 128, :])
                    rhs = hstrip(t)
                    for mc in range(MC):
                        for qc in range(NQ):
                            nc.tensor.matmul(
                                psq[mc * NQ + qc],
                                w[:, mc * 128:(mc + 1) * 128],
                                rhs[:, qc * 512:(qc + 1) * 512],
                                start=(t == 0), stop=(t == HT - 1),
                            )
                for mc in range(MC):
                    for qc in range(NQ):
                        sink(mc, qc, psq[mc * NQ + qc])

            def sink_proj(dst):
                def s(mc, qc, ps):
                    out_ap = dst[mc][:, qc * 512:(qc + 1) * 512]
                    if qc % 2 == 0:
                        nc.scalar.copy(out_ap, ps)
                    else:
                        nc.vector.tensor_copy(out_ap, ps)
                return s

            def sink_out(odram):
                def s(mc, qc, ps):
                    ot = opool.tile([128, 512], F16, tag="ot", name="ot")
                    if qc % 2 == 0:
                        nc.scalar.copy(ot, ps)
                    else:
                        nc.vector.tensor_copy(ot, ps)
                    nc.sync.dma_start(
                        out=odram[mc * 128:(mc + 1) * 128,
                                  qc * 512:(qc + 1) * 512],
                        in_=ot,
                    )
                return s

            # indexer projections first so the score phase can start early;
            # attention projections after (their ACT/DVE copies overlap scores)
            proj_pass(wfq, wfq_r, sink_proj(qpt))
            proj_pass(wfk, wfk_r, sink_proj(kpt))

            # indexer scores: rel_c[q] = sum_k relu(qp_c[q] . kp_c[k])
            relmat = rmpool.tile([128, QT], F32, name="relmat")
            for qt in range(QT):
                spss = [
                    pspool.tile([128, 512], F32, tag=f"m{kc}", name="sps")
                    for kc in range(NQ)
                ]
                for dcc in range(DC):
                    for kc in range(NQ):
                        nc.tensor.matmul(
                            spss[kc],
                            qpt[dcc][:, qt * 128:(qt + 1) * 128],
                            kpt[dcc][:, kc * 512:(kc + 1) * 512],
                            start=(dcc == 0), stop=(dcc == DC - 1),
                        )
                relcols = rcpool.tile([128, NQ + 1], F32, tag="relcols",
                                      name="relcols")
                for kc in (0, 1):  # ACT half: fused relu + free-axis accum
                    scratch = scpool.tile([128, 512], F16, tag=f"scr{kc}",
                                          name="scratch")
                    nc.scalar.activation(
                        scratch, spss[kc], AF.Relu,
                        accum_out=relcols[:, kc:kc + 1],
                    )
                for kc in (2, 3):  # DVE half: relu then reduce
                    scratch = scpool.tile([128, 512], F16, tag=f"scr{kc}",
                                          name="scratch")
                    nc.vector.tensor_scalar_max(scratch, spss[kc], 0.0)
                    nc.vector.tensor_reduce(
                        relcols[:, kc:kc + 1], scratch,
                        axis=mybir.AxisListType.X, op=mybir.AluOpType.add,
                    )
                nc.vector.tensor_reduce(
                    relmat[:, qt:qt + 1], relcols[:, 0:NQ],
                    axis=mybir.AxisListType.X, op=mybir.AluOpType.add,
                )
            nc.sync.dma_start(
                out=rel.rearrange("(t p) -> p t", p=128), in_=relmat
            )

            # attention projections: all fp16 for launch B
            proj_pass(wq, None, sink_out(qT))
            proj_pass(wk, None, sink_out(kT))
            proj_pass(wv, None, sink_out(vT))
    nc.compile()
    return nc


def build_attn(S=2048, H=HIDDEN, NHC=NUM_HEADS // N_CORES, HD=HEAD_DIM,
               window=LOCAL_WINDOW):
    """Launch B: per-core (2 heads) masked softmax attention + out-proj."""
    nc = bacc.Bacc("TRN2", target_bir_lowering=False, debug=False)
    KC, NQ, QT, OCC = S // 128, S // 512, S // 128, H // 512
    qTh = nc.dram_tensor("qTh", [NHC * HD, S], F16, kind="ExternalInput")
    kTh = nc.dram_tensor("kTh", [NHC * HD, S], F16, kind="ExternalInput")
    vTh = nc.dram_tensor("vTh", [NHC * HD, S], F16, kind="ExternalInput")
    woh = nc.dram_tensor("woh", [NHC * HD, H], F16, kind="ExternalInput")
    kidx = nc.dram_tensor("kidx", [S], F16, kind="ExternalInput")
    hivec = nc.dram_tensor("hivec", [S], F16, kind="ExternalInput")
    selv = nc.dram_tensor("selv", [S], F16, kind="ExternalInput")
    onesrow = nc.dram_tensor("onesrow", [128], F32R, kind="ExternalInput")
    part = nc.dram_tensor("part", [S, H], F16, kind="ExternalOutput")

    scale = 1.0 / math.sqrt(HD)
    AF = mybir.ActivationFunctionType
    OP = mybir.AluOpType

    with TileContext(nc) as tc:
        with (
            tc.tile_pool(name="const", bufs=1) as cpool,
            tc.tile_pool(name="qk", bufs=1) as qkpool,
            tc.tile_pool(name="vt", bufs=2) as vtpool,
            tc.tile_pool(name="vh", bufs=1) as vhpool,
            tc.tile_pool(name="vsl", bufs=1) as vslpool,
            tc.tile_pool(name="et", bufs=2) as etpool,
            tc.tile_pool(name="aon", bufs=1) as aopool,
            tc.tile_pool(name="dr", bufs=2) as drpool,
            tc.tile_pool(name="ev", bufs=4) as evpool,
            tc.tile_pool(name="ps", bufs=1, space="PSUM") as pspool,
        ):
            iota = cpool.tile([128, S], F16, name="iota")
            nc.gpsimd.iota(
                iota, pattern=[[1, S]], base=0, channel_multiplier=0,
                allow_small_or_imprecise_dtypes=True,
            )
            ones = cpool.tile([128, 1], F16, name="ones")
            nc.vector.memset(ones, 1.0)
            ident = cpool.tile([128, 128], F16, name="ident")
            make_identity(nc, ident)
            kvec = cpool.tile([128, KC], F16, name="kvec")
            nc.sync.dma_start(out=kvec, in_=kidx.rearrange("(t p) -> p t", p=128))
            hvec = cpool.tile([128, KC], F16, name="hvec")
            nc.sync.dma_start(out=hvec, in_=hivec.rearrange("(t p) -> p t", p=128))
            svec = cpool.tile([128, KC], F16, name="svec")
            nc.sync.dma_start(out=svec, in_=selv.rearrange("(t p) -> p t", p=128))
            svec32 = cpool.tile([128, KC], F32, name="svec32")
            nc.vector.tensor_copy(svec32, svec)
            ones1 = cpool.tile([1, 128], F32R, name="ones1")
            nc.sync.dma_start(out=ones1, in_=onesrow[None, :])

            # head-0 working set first so PE can start early; wo weights last.
            vts0 = vtpool.tile([128, S], F16, tag="vts", name="vts")
            nc.sync.dma_start(out=vts0, in_=vTh[0:HD, :])
            qsb, ksb = [], []
            for h in range(NHC):
                q = qkpool.tile([128, S], F16, name=f"qsb{h}")
                nc.sync.dma_start(out=q, in_=qTh[h * HD:(h + 1) * HD, :])
                qsb.append(q)
                k = qkpool.tile([128, S], F16, name=f"ksb{h}")
                nc.sync.dma_start(out=k, in_=kTh[h * HD:(h + 1) * HD, :])
                ksb.append(k)

            aon = [aopool.tile([128, S], F16, name=f"aon{h}") for h in range(NHC)]
            vhf = [vhpool.tile([128, S], F16, name=f"vhf{h}") for h in range(NHC)]

            for h in range(NHC):
                if h == 0:
                    vts = vts0
                else:
                    vts = vtpool.tile([128, S], F16, tag="vts", name="vts")
                    nc.sync.dma_start(out=vts, in_=vTh[h * HD:(h + 1) * HD, :])
                for kc in range(KC):
                    tp = pspool.tile([128, 128], F16, tag="sc", bufs=3, name="tp")
                    nc.tensor.transpose(tp, vts[:, kc * 128:(kc + 1) * 128], ident)
                    dst = vhf[h][:, kc * 128:(kc + 1) * 128]
                    if kc % 2 == 0:
                        nc.scalar.copy(dst, tp)
                    else:
                        nc.vector.tensor_copy(dst, tp)
                # v pre-multiplied by the selected mask: beyond-local tiles use
                # it as the stationary operand, making masking free there.
                vsl = vslpool.tile([128, S], F16, name=f"vsl{h}")
                for kc in range(KC):
                    nc.vector.tensor_scalar_mul(
                        vsl[:, kc * 128:(kc + 1) * 128],
                        vhf[h][:, kc * 128:(kc + 1) * 128],
                        svec32[:, kc:kc + 1],
                    )
                # kc-outer: stationary operands (k tile, v tile) reused across
                # the q chunks; av/den accumulate per q chunk across kc.
                avp = [
                    pspool.tile([128, 512], F32, tag=f"av{qc}", bufs=1,
                                name=f"av{qc}")
                    for qc in range(NQ)
                ]
                den128 = pspool.tile([128, 512], F32, tag="den", bufs=1,
                                     name="den128")
                ets = {}
                for kc in range(KC):
                    k0 = kc * 128
                    qcs = [qc for qc in range(NQ) if k0 <= qc * 512 + 511]
                    far = {qc: qc * 512 > k0 + 127 + window for qc in qcs}
                    for qc in qcs:
                        q0 = qc * 512
                        q1 = q0 + 511
                        sps = pspool.tile([128, 512], F32, tag="sc", bufs=3,
                                          name="sps")
                        nc.tensor.matmul(
                            sps, ksb[h][:, kc * 128:(kc + 1) * 128],
                            qsb[h][:, q0:q0 + 512], start=True, stop=True,
                        )
                        et = etpool.tile([128, 512], F16, tag=f"et{qc}",
                                         name=f"et{qc}")
                        ets[qc] = et
                        nc.scalar.activation(et, sps, AF.Exp, scale=scale)
                        if far[qc]:
                            continue  # sel-mask folded into vsl/svec operands
                        if q0 < k0 + 128:
                            # causal: zero where q < k (iota - k < 0)
                            nc.gpsimd.affine_select(
                                out=et, in_=et, compare_op=OP.is_ge, fill=0.0,
                                base=q0 - k0, channel_multiplier=-1,
                                pattern=[[1, 512]],
                            )
                        if q1 > k0 + window:
                            nc.vector.scalar_tensor_tensor(
                                et, iota[:, q0:q0 + 512], hvec[:, kc:kc + 1], et,
                                op0=OP.is_le, op1=OP.mult,
                            )
                    for qc in qcs:
                        lhs_av = vsl if far[qc] else vhf[h]
                        nc.tensor.matmul(
                            avp[qc], lhs_av[:, kc * 128:(kc + 1) * 128], ets[qc],
                            start=(kc == 0), stop=(kc == (qc * 512 + 511) // 128),
                        )
                    for qc in qcs:
                        lhs_den = svec[:, kc:kc + 1] if far[qc] else ones
                        nc.tensor.matmul(
                            den128[32 * qc:32 * qc + 1, :], lhs_den, ets[qc],
                            start=(kc == 0), stop=(kc == (qc * 512 + 511) // 128),
                            tile_position=(0, 32 * qc),
                        )
                # denominators -> reciprocals -> broadcast -> normalize
                for qc in range(NQ):
                    q0 = qc * 512
                    dq = drpool.tile([1, 512], F32, tag=f"dq{qc}", name=f"dq{qc}")
                    nc.scalar.copy(dq, den128[32 * qc:32 * qc + 1, :])
                    rq = drpool.tile([1, 512], F32, tag=f"rq{qc}", name=f"rq{qc}")
                    rs = drpool.tile([1, 512], F32, tag=f"rs{qc}", name=f"rs{qc}")
                    nc.vector.reciprocal_approx_accurate(rq, dq, rs)
                    rcq = drpool.tile([1, 512], F32R, tag=f"rcq{qc}",
                                      name=f"rcq{qc}")
                    nc.vector.tensor_copy(rcq, rq)
                    rb = pspool.tile([128, 512], F32, tag="sc", bufs=3, name="rb")
                    nc.tensor.matmul(rb, ones1, rcq, start=True, stop=True)
                    rbs = drpool.tile([128, 512], F32, tag="rbs", name="rbs")
                    nc.vector.tensor_copy(rbs, rb)
                    nc.vector.scalar_tensor_tensor(
                        aon[h][:, q0:q0 + 512], rbs, 1.0, avp[qc],
                        op0=OP.mult, op1=OP.mult,
                    )
            wsb = []
            for h in range(NHC):
                w = qkpool.tile([128, H], F16, name=f"wsb{h}")
                nc.sync.dma_start(out=w, in_=woh[h * HD:(h + 1) * HD, :])
                wsb.append(w)
            for qt in range(QT):
                wops = [
                    pspool.tile([128, 512], F32, tag=f"av{oc}", bufs=1,
                                name=f"wops{oc}")
                    for oc in range(OCC)
                ]
                for h in range(NHC):
                    for oc in range(OCC):
                        nc.tensor.matmul(
                            wops[oc], aon[h][:, qt * 128:(qt + 1) * 128],
                            wsb[h][:, oc * 512:(oc + 1) * 512],
                            start=(h == 0), stop=(h == NHC - 1),
                        )
                for oc in range(OCC):
                    ot = evpool.tile([128, 512], F16, tag="ot", name="ot")
                    if oc % 2 == 0:
                        nc.scalar.copy(ot, wops[oc])
                    else:
                        nc.vector.tensor_copy(ot, wops[oc])
                    nc.sync.dma_start(
                        out=part[qt * 128:(qt + 1) * 128, oc * 512:(oc + 1) * 512],
                        in_=ot,
                    )
    nc.compile()
    return nc


_CACHE = {}


def _get(name, builder, *args):
    key = (name,) + args
    if key not in _CACHE:
        _CACHE[key] = builder(*args)
    return _CACHE[key]


def _run(nc, in_maps):
    res = run_bass_kernel_spmd(
        nc, in_maps, core_ids=list(range(N_CORES)), trace=_TRACE["on"]
    )
    if _TRACE["on"] and res.exec_time_ns is not None:
        _TRACE["exec_ns"].append(res.exec_time_ns)
    return res.results


def kernel(hidden_states, Wq, Wk, Wv, Wo, Wq_ind, Wk_ind, head_weights,
           temperature_param):
    hidden_states = np.asarray(hidden_states, dtype=FP32)
    Wq, Wk, Wv, Wo = (np.asarray(a, dtype=FP32) for a in (Wq, Wk, Wv, Wo))
    Wq_ind = np.asarray(Wq_ind, dtype=FP32)
    Wk_ind = np.asarray(Wk_ind, dtype=FP32)
    head_weights = np.asarray(head_weights, dtype=FP32)

    B, S, H = hidden_states.shape
    assert B == 1 and H == HIDDEN
    CS = H // N_CORES
    D = IND_DIM
    hidT = np.ascontiguousarray(hidden_states[0].T)
    Wfq = Wq @ Wq_ind  # fused indexer weights (f32 host fuse)
    Wfk = Wk @ Wk_ind

    # ---- Launch A: projections + indexer, head-parallel ----
    ncA = _get("A", build_fused, S, H, CS, D)
    inA = [
        {
            "hidT": hidT,
            "wq": np.ascontiguousarray(Wq[:, c * CS:(c + 1) * CS]),
            "wk": np.ascontiguousarray(Wk[:, c * CS:(c + 1) * CS]),
            "wv": np.ascontiguousarray(Wv[:, c * CS:(c + 1) * CS]),
            "wfq": np.ascontiguousarray(Wfq[:, c * D:(c + 1) * D]),
            "wfk": np.ascontiguousarray(Wfk[:, c * D:(c + 1) * D]),
        }
        for c in range(N_CORES)
    ]
    rA = _run(ncA, inA)
    rel = np.zeros(S, dtype=np.float64)
    for c in range(N_CORES):
        rel += float(head_weights[c]) * rA[c]["rel"].astype(np.float64)
    # exp(-temp) scaling is monotone; irrelevant for top-k selection.

    k_sel = min(MAX_SELECTED, S)
    top_idx = np.argpartition(-rel, k_sel - 1)[:k_sel]
    selected = np.zeros(S, dtype=bool)
    selected[top_idx] = True

    # ---- Launch B: masked attention + output projection, head-parallel ----
    BIG = float(2 * S + 1024)
    hi = np.where(selected, BIG, np.arange(S, dtype=np.float64) + LOCAL_WINDOW)
    hi = hi.astype(np.float16)
    kidx = np.arange(S, dtype=np.float16)
    selv = selected.astype(np.float16)
    NHC = NUM_HEADS // N_CORES
    RW = NHC * HEAD_DIM
    ncB = _get("B", build_attn, S, H, NHC, HEAD_DIM, LOCAL_WINDOW)
    inB = [
        {
            "qTh": rA[c]["qT"],
            "kTh": rA[c]["kT"],
            "vTh": rA[c]["vT"],
            "woh": np.ascontiguousarray(Wo[c * RW:(c + 1) * RW]).astype(
                np.float16),
            "kidx": kidx,
            "hivec": hi,
            "selv": selv,
            "onesrow": np.ones(128, dtype=np.float32),
        }
        for c in range(N_CORES)
    ]
    rB = _run(ncB, inB)
    out = rB[0]["part"].astype(np.float32)
    for c in range(1, N_CORES):
        out += rB[c]["part"].astype(np.float32)
    return out.reshape(B, S, H)


# revision 25
# speedup vs baseline: 1.3325x; 1.0759x over previous
"""DeepSeek sparse attention on 8 Trainium2 NeuronCores (Bass/Tile).

Strategy (2 SPMD launches, head-parallel, f32r indexer + fp16 attention):

  host: fuse indexer weights through the attention projections:
      Wfq = Wq @ Wq_ind, Wfk = Wk @ Wk_ind  (so the indexer reads
      hidden directly and needs no q_lin/k_lin round trip).
  A   (fused projections + indexer): core c keeps hidden^T resident in
      SBUF (one 16.8 MB stream) and runs five accumulation passes:
      wfq -> qp_c^T, wfk -> kp_c^T (indexer head c, f32r), then the
      relu(qp.kp) score reduction -> rel_c, then wq/wk/wv -> the
      column slices qT/kT (fp16 out) / vT (f32 out) = exactly the
      core's own 2 attention heads.
  host: rel = sum_c w_c * rel_c; top-1024 keys -> selected mask ->
      hi[k] threshold vector (fp16).
  B   (attention, fp16): core c computes softmax attention for its 2
      heads from its own qT/kT/vT slices, masked causal/local/selected
      via iota-compare DVE ops, then partial out rows = ao @ Wo[rows]
      -> per-core partial (S, H) in fp16.
  host: out = sum_c partial_c.

The indexer path stays f32r end-to-end: the top-k boundary gaps are
~1e-5 relative, and selection flips cost ~1e-2 output error each.
The attention path is fp16 (storage) with f32 PE accumulation.
"""

import math

import numpy as np

import concourse.bass as bass
import concourse.mybir as mybir
from concourse import bacc
from concourse.tile import TileContext
from concourse.masks import make_identity
from concourse.bass_utils import run_bass_kernel_spmd

# Problem constants (hardcoded per contract)
HIDDEN = 2048
NUM_HEADS = 16
HEAD_DIM = 128
NUM_IND_HEADS = 8
IND_DIM = HIDDEN // NUM_IND_HEADS  # 256
MAX_SELECTED = 1024
LOCAL_WINDOW = 512
N_CORES = 8

F32 = mybir.dt.float32
F32R = mybir.dt.float32r
F16 = mybir.dt.float16
FP32 = np.float32

_TRACE = {"on": False, "exec_ns": []}


def build_fused(S=2048, H=HIDDEN, CS=HIDDEN // N_CORES, D=IND_DIM):
    """Launch A: hidden^T resident; 5 projection passes + indexer scores."""
    nc = bacc.Bacc("TRN2", target_bir_lowering=False, debug=False)
    HT, NQ, QT, DC, MC = H // 128, S // 512, S // 128, D // 128, CS // 128
    # all inputs arrive partition-major (host pre-rearranged): x[p, t*C+c]
    # = orig[t*128+p, c] — plain 2D DMAs, 128 descriptors each.
    hidT = nc.dram_tensor("hidT", [128, HT * S], F32R, kind="ExternalInput")
    wq = nc.dram_tensor("wq", [128, HT * CS], F32R, kind="ExternalInput")
    wk = nc.dram_tensor("wk", [128, HT * CS], F32R, kind="ExternalInput")
    wv = nc.dram_tensor("wv", [128, HT * CS], F32R, kind="ExternalInput")
    wfq = nc.dram_tensor("wfq", [128, HT * D], F32R, kind="ExternalInput")
    wfk = nc.dram_tensor("wfk", [128, HT * D], F32R, kind="ExternalInput")
    qT = nc.dram_tensor("qT", [CS, S], F16, kind="ExternalOutput")
    kT = nc.dram_tensor("kT", [CS, S], F16, kind="ExternalOutput")
    vT = nc.dram_tensor("vT", [CS, S], F16, kind="ExternalOutput")
    rel = nc.dram_tensor("rel", [S], F32, kind="ExternalOutput")

    AF = mybir.ActivationFunctionType

    with TileContext(nc) as tc:
        with (
            tc.tile_pool(name="hid", bufs=1) as hpool,
            tc.tile_pool(name="wres", bufs=1) as wrpool,
            tc.tile_pool(name="wt", bufs=2) as wpool,
            tc.tile_pool(name="proj", bufs=1) as ppool,
            tc.tile_pool(name="ev", bufs=4) as opool,
            tc.tile_pool(name="scr", bufs=1) as scpool,
            tc.tile_pool(name="rc", bufs=2) as rcpool,
            tc.tile_pool(name="rm", bufs=1) as rmpool,
            tc.tile_pool(name="ps", bufs=1, space="PSUM") as pspool,
        ):
            # hidden^T resident, loaded as 8 chunks of 2 k-strips so the first
            # matmuls only wait on chunk 0 (~2 MB), not the full 16 MB.
            G = 8
            TG = HT // G

            def load_hidc(g):
                hc = hpool.tile([128, TG * S], F32R, name=f"hidc{g}")
                nc.sync.dma_start(out=hc, in_=hidT[:, g * TG * S:(g + 1) * TG * S])
                return hc

            def load_wres(wd):
                # indexer weight resident: one 2 MB DMA, issued early so it
                # is not queued behind the 16 MB hidden stream.
                wr = wrpool.tile([128, HT * D], F32R, tag=wd.name, name=wd.name)
                nc.sync.dma_start(out=wr, in_=wd[:, :])
                return wr

            WG = 4  # weight strips per streamed DMA group

            def load_wgroup(wd, g):
                w = wpool.tile([128, WG * CS], F32R, tag="ws", name="ws")
                nc.sync.dma_start(
                    out=w, in_=wd[:, g * WG * CS:(g + 1) * WG * CS])
                return w

            # DMA issue order: chunk0, wfq, chunks1-7, wfk groups — pass 1
            # (wfq) operands land early; wfk starts arriving right as the
            # full hidden lands (when the fk pass begins).
            hidc = [load_hidc(0)]
            wfq_r = load_wres(wfq)
            hidc += [load_hidc(g) for g in range(1, G)]
            wfk_groups = [load_wgroup(wfk, 0), load_wgroup(wfk, 1), None, None]

            def hstrip(t):
                return hidc[t // TG][:, (t % TG) * S:(t % TG) * S + S]

            qpt = [ppool.tile([128, S], F32R, name=f"qpt{i}") for i in range(DC)]
            kpt = [ppool.tile([128, S], F32R, name=f"kpt{i}") for i in range(DC)]

            def proj_pass(wd, wres, sink, wgroups=None):
                psq = [
                    pspool.tile([128, 512], F32, tag=f"m{i}", name=f"m{i}")
                    for i in range(MC * NQ)
                ]
                for t in range(HT):
                    if wres is not None:
                        w = wres[:, t * CS:(t + 1) * CS]
                    else:
                        g, tl = t // WG, t % WG
                        if wgroups[g] is None:
                            wgroups[g] = load_wgroup(wd, g)
                        w = wgroups[g][:, tl * CS:(tl + 1) * CS]
                    rhs = hstrip(t)
                    for mc in range(MC):
                        for qc in range(NQ):
                            nc.tensor.matmul(
                                psq[mc * NQ + qc],
                                w[:, mc * 128:(mc + 1) * 128],
                                rhs[:, qc * 512:(qc + 1) * 512],
                                start=(t == 0), stop=(t == HT - 1),
                            )
                for mc in range(MC):
                    for qc in range(NQ):
                        sink(mc, qc, psq[mc * NQ + qc])

            def sink_proj(dst):
                def s(mc, qc, ps):
                    out_ap = dst[mc][:, qc * 512:(qc + 1) * 512]
                    if qc % 2 == 0:
                        nc.scalar.copy(out_ap, ps)
                    else:
                        nc.vector.tensor_copy(out_ap, ps)
                return s

            def sink_out(odram):
                def s(mc, qc, ps):
                    ot = opool.tile([128, 512], F16, tag="ot", name="ot")
                    if qc % 2 == 0:
                        nc.scalar.copy(ot, ps)
                    else:
                        nc.vector.tensor_copy(ot, ps)
                    nc.sync.dma_start(
                        out=odram[mc * 128:(mc + 1) * 128,
                                  qc * 512:(qc + 1) * 512],
                        in_=ot,
                    )
                return s

            # indexer projections first so the score phase can start early;
            # attention projections after (their ACT/DVE copies overlap scores)
            proj_pass(wfq, wfq_r, sink_proj(qpt))
            proj_pass(wfk, None, sink_proj(kpt), wfk_groups)

            # prefetch the q-pass weights so the PE does not stall on them
            # when the score phase ends
            wq_groups = [load_wgroup(wq, 0), load_wgroup(wq, 1), None, None]

            # indexer scores: rel_c[q] = sum_k relu(qp_c[q] . kp_c[k])
            relmat = rmpool.tile([128, QT], F32, name="relmat")
            for qt in range(QT):
                spss = [
                    pspool.tile([128, 512], F32, tag=f"m{kc}", name="sps")
                    for kc in range(NQ)
                ]
                for dcc in range(DC):
                    for kc in range(NQ):
                        nc.tensor.matmul(
                            spss[kc],
                            qpt[dcc][:, qt * 128:(qt + 1) * 128],
                            kpt[dcc][:, kc * 512:(kc + 1) * 512],
                            start=(dcc == 0), stop=(dcc == DC - 1),
                        )
                relcols = rcpool.tile([128, NQ + 1], F32, tag="relcols",
                                      name="relcols")
                # relu+reduce split ~2.5/1.5 between ACT and DVE
                act_kcs = (0, 1, 2) if qt % 2 == 0 else (0, 1)
                for kc in range(NQ):
                    scratch = scpool.tile([128, 512], F16,
                                          tag=f"scr{kc in act_kcs}",
                                          name="scratch")
                    if kc in act_kcs:  # ACT: fused relu + free-axis accum
                        nc.scalar.activation(
                            scratch, spss[kc], AF.Relu,
                            accum_out=relcols[:, kc:kc + 1],
                        )
                    else:  # DVE: relu then reduce
                        nc.vector.tensor_scalar_max(scratch, spss[kc], 0.0)
                        nc.vector.tensor_reduce(
                            relcols[:, kc:kc + 1], scratch,
                            axis=mybir.AxisListType.X, op=mybir.AluOpType.add,
                        )
                nc.vector.tensor_reduce(
                    relmat[:, qt:qt + 1], relcols[:, 0:NQ],
                    axis=mybir.AxisListType.X, op=mybir.AluOpType.add,
                )
            nc.sync.dma_start(
                out=rel.rearrange("(t p) -> p t", p=128), in_=relmat
            )

            # attention projections: all fp16 for launch B
            proj_pass(wq, None, sink_out(qT), wq_groups)
            proj_pass(wk, None, sink_out(kT), [None] * (HT // WG))
            proj_pass(wv, None, sink_out(vT), [None] * (HT // WG))
    nc.compile()
    return nc


def build_attn(S=2048, H=HIDDEN, NHC=NUM_HEADS // N_CORES, HD=HEAD_DIM,
               window=LOCAL_WINDOW):
    """Launch B: per-core (2 heads) masked softmax attention + out-proj."""
    nc = bacc.Bacc("TRN2", target_bir_lowering=False, debug=False)
    KC, NQ, QT, OCC = S // 128, S // 512, S // 128, H // 512
    qTh = nc.dram_tensor("qTh", [NHC * HD, S], F16, kind="ExternalInput")
    kTh = nc.dram_tensor("kTh", [NHC * HD, S], F16, kind="ExternalInput")
    vTh = nc.dram_tensor("vTh", [NHC * HD, S], F16, kind="ExternalInput")
    woh = nc.dram_tensor("woh", [NHC * HD, H], F16, kind="ExternalInput")
    kidx = nc.dram_tensor("kidx", [S], F16, kind="ExternalInput")
    hivec = nc.dram_tensor("hivec", [S], F16, kind="ExternalInput")
    selv = nc.dram_tensor("selv", [S], F16, kind="ExternalInput")
    onesrow = nc.dram_tensor("onesrow", [128], F32R, kind="ExternalInput")
    part = nc.dram_tensor("part", [S, H], F16, kind="ExternalOutput")

    scale = 1.0 / math.sqrt(HD)
    AF = mybir.ActivationFunctionType
    OP = mybir.AluOpType

    with TileContext(nc) as tc:
        with (
            tc.tile_pool(name="const", bufs=1) as cpool,
            tc.tile_pool(name="qk", bufs=1) as qkpool,
            tc.tile_pool(name="vt", bufs=2) as vtpool,
            tc.tile_pool(name="vh", bufs=1) as vhpool,
            tc.tile_pool(name="vsl", bufs=1) as vslpool,
            tc.tile_pool(name="et", bufs=2) as etpool,
            tc.tile_pool(name="aon", bufs=1) as aopool,
            tc.tile_pool(name="dr", bufs=2) as drpool,
            tc.tile_pool(name="ev", bufs=4) as evpool,
            tc.tile_pool(name="ps", bufs=1, space="PSUM") as pspool,
        ):
            # head-0 working set first so PE can start early.
            vts0 = vtpool.tile([128, S], F16, tag="vts", name="vts")
            nc.sync.dma_start(out=vts0, in_=vTh[0:HD, :])
            qsb, ksb = [], []
            for h in range(NHC):
                q = qkpool.tile([128, S], F16, name=f"qsb{h}")
                nc.sync.dma_start(out=q, in_=qTh[h * HD:(h + 1) * HD, :])
                qsb.append(q)
                k = qkpool.tile([128, S], F16, name=f"ksb{h}")
                nc.sync.dma_start(out=k, in_=kTh[h * HD:(h + 1) * HD, :])
                ksb.append(k)

            iota = cpool.tile([128, S], F16, name="iota")
            nc.gpsimd.iota(
                iota, pattern=[[1, S]], base=0, channel_multiplier=0,
                allow_small_or_imprecise_dtypes=True,
            )
            ones = cpool.tile([128, 1], F16, name="ones")
            nc.vector.memset(ones, 1.0)
            ident = cpool.tile([128, 128], F16, name="ident")
            make_identity(nc, ident)
            kvec = cpool.tile([128, KC], F16, name="kvec")
            nc.sync.dma_start(out=kvec, in_=kidx.rearrange("(t p) -> p t", p=128))
            hvec = cpool.tile([128, KC], F16, name="hvec")
            nc.sync.dma_start(out=hvec, in_=hivec.rearrange("(t p) -> p t", p=128))
            svec = cpool.tile([128, KC], F16, name="svec")
            nc.sync.dma_start(out=svec, in_=selv.rearrange("(t p) -> p t", p=128))
            svec32 = cpool.tile([128, KC], F32, name="svec32")
            nc.vector.tensor_copy(svec32, svec)
            ones1 = cpool.tile([1, 128], F32R, name="ones1")
            nc.sync.dma_start(out=ones1, in_=onesrow[None, :])

            # out-proj weights early so the final phase never stalls on them
            wsb = []
            for h in range(NHC):
                w = qkpool.tile([128, H], F16, name=f"wsb{h}")
                nc.sync.dma_start(out=w, in_=woh[h * HD:(h + 1) * HD, :])
                wsb.append(w)

            aon = [aopool.tile([128, S], F16, name=f"aon{h}") for h in range(NHC)]
            vhf = [vhpool.tile([128, S], F16, name=f"vhf{h}") for h in range(NHC)]

            for h in range(NHC):
                if h == 0:
                    vts = vts0
                else:
                    vts = vtpool.tile([128, S], F16, tag="vts", name="vts")
                    nc.sync.dma_start(out=vts, in_=vTh[h * HD:(h + 1) * HD, :])
                for kc in range(KC):
                    tp = pspool.tile([128, 128], F16, tag="sc", bufs=3, name="tp")
                    nc.tensor.transpose(tp, vts[:, kc * 128:(kc + 1) * 128], ident)
                    dst = vhf[h][:, kc * 128:(kc + 1) * 128]
                    if kc % 2 == 0:
                        nc.scalar.copy(dst, tp)
                    else:
                        nc.vector.tensor_copy(dst, tp)
                # v pre-multiplied by the selected mask: beyond-local tiles use
                # it as the stationary operand, making masking free there.
                vsl = vslpool.tile([128, S], F16, name=f"vsl{h}")
                for kc in range(KC):
                    nc.vector.tensor_scalar_mul(
                        vsl[:, kc * 128:(kc + 1) * 128],
                        vhf[h][:, kc * 128:(kc + 1) * 128],
                        svec32[:, kc:kc + 1],
                    )
                # kc-outer: stationary operands (k tile, v tile) reused across
                # the q chunks; av/den accumulate per q chunk across kc.
                avp = [
                    pspool.tile([128, 512], F32, tag=f"av{qc}", bufs=1,
                                name=f"av{qc}")
                    for qc in range(NQ)
                ]
                den128 = pspool.tile([128, 512], F32, tag="den", bufs=1,
                                     name="den128")
                ets = {}
                for kc in range(KC):
                    k0 = kc * 128
                    qcs = [qc for qc in range(NQ) if k0 <= qc * 512 + 511]
                    far = {qc: qc * 512 > k0 + 127 + window for qc in qcs}
                    for qc in qcs:
                        q0 = qc * 512
                        q1 = q0 + 511
                        sps = pspool.tile([128, 512], F32, tag="sc", bufs=3,
                                          name="sps")
                        nc.tensor.matmul(
                            sps, ksb[h][:, kc * 128:(kc + 1) * 128],
                            qsb[h][:, q0:q0 + 512], start=True, stop=True,
                        )
                        et = etpool.tile([128, 512], F16, tag=f"et{qc}",
                                         name=f"et{qc}")
                        ets[qc] = et
                        nc.scalar.activation(et, sps, AF.Exp, scale=scale)
                        if far[qc]:
                            continue  # sel-mask folded into vsl/svec operands
                        if q0 < k0 + 128:
                            # causal: zero where q < k (iota - k < 0)
                            nc.gpsimd.affine_select(
                                out=et, in_=et, compare_op=OP.is_ge, fill=0.0,
                                base=q0 - k0, channel_multiplier=-1,
                                pattern=[[1, 512]],
                            )
                        if q1 > k0 + window:
                            nc.vector.scalar_tensor_tensor(
                                et, iota[:, q0:q0 + 512], hvec[:, kc:kc + 1], et,
                                op0=OP.is_le, op1=OP.mult,
                            )
                    for qc in qcs:
                        lhs_av = vsl if far[qc] else vhf[h]
                        nc.tensor.matmul(
                            avp[qc], lhs_av[:, kc * 128:(kc + 1) * 128], ets[qc],
                            start=(kc == 0), stop=(kc == (qc * 512 + 511) // 128),
                        )
                    for qc in qcs:
                        lhs_den = svec[:, kc:kc + 1] if far[qc] else ones
                        nc.tensor.matmul(
                            den128[32 * qc:32 * qc + 1, :], lhs_den, ets[qc],
                            start=(kc == 0), stop=(kc == (qc * 512 + 511) // 128),
                            tile_position=(0, 32 * qc),
                        )
                # denominators: broadcast via PE first (one short hop off the
                # den psum), then reciprocal + normalize on DVE — the PE never
                # waits on the DVE chain.
                for qc in range(NQ):
                    q0 = qc * 512
                    dq = drpool.tile([1, 512], F32R, tag=f"dq{qc}", name=f"dq{qc}")
                    nc.scalar.copy(dq, den128[32 * qc:32 * qc + 1, :])
                    rb = pspool.tile([128, 512], F32, tag="sc", bufs=3, name="rb")
                    nc.tensor.matmul(rb, ones1, dq, start=True, stop=True)
                    rbs = drpool.tile([128, 512], F32, tag="rbs", name="rbs")
                    rs = drpool.tile([128, 512], F32, tag="rs", name="rs")
                    nc.vector.reciprocal_approx_accurate(rbs, rb, rs)
                    nc.vector.scalar_tensor_tensor(
                        aon[h][:, q0:q0 + 512], rbs, 1.0, avp[qc],
                        op0=OP.mult, op1=OP.mult,
                    )
            for qt in range(QT):
                wops = [
                    pspool.tile([128, 512], F32, tag=f"av{oc}", bufs=1,
                                name=f"wops{oc}")
                    for oc in range(OCC)
                ]
                for h in range(NHC):
                    for oc in range(OCC):
                        nc.tensor.matmul(
                            wops[oc], aon[h][:, qt * 128:(qt + 1) * 128],
                            wsb[h][:, oc * 512:(oc + 1) * 512],
                            start=(h == 0), stop=(h == NHC - 1),
                        )
                for oc in range(OCC):
                    ot = evpool.tile([128, 512], F16, tag="ot", name="ot")
                    if oc % 2 == 0:
                        nc.scalar.copy(ot, wops[oc])
                    else:
                        nc.vector.tensor_copy(ot, wops[oc])
                    nc.sync.dma_start(
                        out=part[qt * 128:(qt + 1) * 128, oc * 512:(oc + 1) * 512],
                        in_=ot,
                    )
    nc.compile()
    return nc


_CACHE = {}


def _get(name, builder, *args):
    key = (name,) + args
    if key not in _CACHE:
        _CACHE[key] = builder(*args)
    return _CACHE[key]


def _run(nc, in_maps):
    res = run_bass_kernel_spmd(
        nc, in_maps, core_ids=list(range(N_CORES)), trace=_TRACE["on"]
    )
    if _TRACE["on"] and res.exec_time_ns is not None:
        _TRACE["exec_ns"].append(res.exec_time_ns)
    return res.results


def kernel(hidden_states, Wq, Wk, Wv, Wo, Wq_ind, Wk_ind, head_weights,
           temperature_param):
    hidden_states = np.asarray(hidden_states, dtype=FP32)
    Wq, Wk, Wv, Wo = (np.asarray(a, dtype=FP32) for a in (Wq, Wk, Wv, Wo))
    Wq_ind = np.asarray(Wq_ind, dtype=FP32)
    Wk_ind = np.asarray(Wk_ind, dtype=FP32)
    head_weights = np.asarray(head_weights, dtype=FP32)

    B, S, H = hidden_states.shape
    assert B == 1 and H == HIDDEN
    CS = H // N_CORES
    D = IND_DIM
    HT = H // 128
    Wfq = Wq @ Wq_ind  # fused indexer weights (f32 host fuse)
    Wfk = Wk @ Wk_ind

    def pmajor(x):
        # (H, C) -> (128, HT*C): out[p, t*C+c] = x[t*128+p, c]
        C = x.shape[1]
        return np.ascontiguousarray(
            x.reshape(HT, 128, C).transpose(1, 0, 2).reshape(128, HT * C))

    hidT = pmajor(np.ascontiguousarray(hidden_states[0].T))

    # ---- Launch A: projections + indexer, head-parallel ----
    ncA = _get("A", build_fused, S, H, CS, D)
    inA = [
        {
            "hidT": hidT,
            "wq": pmajor(Wq[:, c * CS:(c + 1) * CS]),
            "wk": pmajor(Wk[:, c * CS:(c + 1) * CS]),
            "wv": pmajor(Wv[:, c * CS:(c + 1) * CS]),
            "wfq": pmajor(Wfq[:, c * D:(c + 1) * D]),
            "wfk": pmajor(Wfk[:, c * D:(c + 1) * D]),
        }
        for c in range(N_CORES)
    ]
    rA = _run(ncA, inA)
    rel = np.zeros(S, dtype=np.float64)
    for c in range(N_CORES):
        rel += float(head_weights[c]) * rA[c]["rel"].astype(np.float64)
    # exp(-temp) scaling is monotone; irrelevant for top-k selection.

    k_sel = min(MAX_SELECTED, S)
    top_idx = np.argpartition(-rel, k_sel - 1)[:k_sel]
    selected = np.zeros(S, dtype=bool)
    selected[top_idx] = True

    # ---- Launch B: masked attention + output projection, head-parallel ----
    BIG = float(2 * S + 1024)
    hi = np.where(selected, BIG, np.arange(S, dtype=np.float64) + LOCAL_WINDOW)
    hi = hi.astype(np.float16)
    kidx = np.arange(S, dtype=np.float16)
    selv = selected.astype(np.float16)
    NHC = NUM_HEADS // N_CORES
    RW = NHC * HEAD_DIM
    ncB = _get("B", build_attn, S, H, NHC, HEAD_DIM, LOCAL_WINDOW)
    inB = [
        {
            "qTh": rA[c]["qT"],
            "kTh": rA[c]["kT"],
            "vTh": rA[c]["vT"],
            "woh": np.ascontiguousarray(Wo[c * RW:(c + 1) * RW]).astype(
                np.float16),
            "kidx": kidx,
            "hivec": hi,
            "selv": selv,
            "onesrow": np.ones(128, dtype=np.float32),
        }
        for c in range(N_CORES)
    ]
    rB = _run(ncB, inB)
    out = rB[0]["part"].astype(np.float32)
    for c in range(1, N_CORES):
        out += rB[c]["part"].astype(np.float32)
    return out.reshape(B, S, H)
